# revision 10
# baseline (speedup 1.0000x reference)
"""Trainium2 Bass kernel for nn_EnhancedGraphConv (gnn_message_passing).

Strategy (8 cores): shard the B*N=1280 graph rows (b,i) as 160 rows/core
(cores 0-3 -> batch 0, 4-7 -> batch 1).  Host converts the dense adjacency
into padded neighbor lists (max degree 51 <= 64 slots/row), and the device
gathers only the ~5% of edge_features / x rows that are actually referenced,
via indirect DMA from HBM.  All per-edge MLPs run on compacted tokens in
feature-major layout; attention softmax runs row-major (rows on partitions,
64 neighbor slots on the free dim); messages are reduced with a PE ones-
broadcast + strided DVE reduction.
"""
import numpy as np
from contextlib import ExitStack

import concourse.bass as bass
import concourse.bacc as bacc
import concourse.tile as tile
from concourse import mybir
from concourse.bass_utils import run_bass_kernel_spmd
from concourse.masks import make_identity

F32 = mybir.dt.float32
I32 = mybir.dt.int32
AF = mybir.ActivationFunctionType
OP = mybir.AluOpType

B, N, C, O, E = 2, 640, 64, 64, 18
D = 64            # neighbor slots per row
RG = 32           # rows per group
NCORES = 8
RPC = (B * N) // NCORES   # 160 rows per core
NG = RPC // RG            # 5 groups
TG = D * RG               # 2048 tokens per group
CHUNK = 512               # matmul moving chunk
NCH = TG // CHUNK         # 4 chunks per group
NCOL = TG // 128          # 16 gather columns per group


def _build_nc(debug=False):
    nc = bacc.Bacc("TRN2", target_bir_lowering=False)
    t = {}
    inp = [
        ("efp", [RPC * N, 64]), ("x", [N, C]), ("xrows", [RPC, C]),
        ("am", [RPC, D]),
        ("We1", [E, 64]), ("We2", [64, 64]), ("We3", [64, 32]),
        ("Wpe", [32, 128]), ("Wjj", [64, 128]), ("Wxi", [64, 64]),
        ("Wn", [64, 64]), ("W22", [128, 128]), ("Wa3", [32, 1]),
        ("Ws", [64, 64]), ("Wc1", [128, 64]), ("Wc2", [64, 64]),
        ("be1", [64, 1]), ("be2", [64, 1]), ("be3", [32, 1]),
        ("bhg", [128, 1]), ("bn", [64, 1]), ("ba2", [32, 1]),
        ("bg2", [64, 1]), ("bs", [64, 1]), ("bc1", [64, 1]), ("bc2", [64, 1]),
    ]
    for name, shape in inp:
        t[name] = nc.dram_tensor(name, shape, F32, kind="ExternalInput")
    I16 = mybir.dt.int16
    t["idxj"] = nc.dram_tensor("idxj", [128, NG * (TG // 16)], I16, kind="ExternalInput")
    t["idxe"] = nc.dram_tensor("idxe", [128, NG * (TG // 16)], I16, kind="ExternalInput")
    t["out"] = nc.dram_tensor("out", [RPC, O], F32, kind="ExternalOutput")
    if debug:
        t["dbg_xj"] = nc.dram_tensor("dbg_xj", [128, NCOL, C], F32, kind="ExternalOutput")
        t["dbg_ef"] = nc.dram_tensor("dbg_ef", [128, NCOL, 64], F32, kind="ExternalOutput")
        t["dbg_wflat"] = nc.dram_tensor("dbg_wflat", [1, TG], F32, kind="ExternalOutput")
        t["dbg_mdw"] = nc.dram_tensor("dbg_mdw", [64, TG], F32, kind="ExternalOutput")
        t["dbg_hg"] = nc.dram_tensor("dbg_hg", [128, TG], F32, kind="ExternalOutput")
        t["dbg_wrow"] = nc.dram_tensor("dbg_wrow", [RG, D], F32, kind="ExternalOutput")

    with tile.TileContext(nc) as tc, ExitStack() as ctx:
        w = ctx.enter_context(tc.tile_pool(name="w", bufs=1))
        big = ctx.enter_context(tc.tile_pool(name="big", bufs=1))
        sm = ctx.enter_context(tc.tile_pool(name="sm", bufs=2))
        ps = ctx.enter_context(tc.tile_pool(name="ps", bufs=3, space="PSUM"))
        pst = ctx.enter_context(tc.tile_pool(name="pst", bufs=1, space="PSUM"))
        pss = ctx.enter_context(tc.tile_pool(name="pss", bufs=2, space="PSUM"))

        # ---- constants / weights in SBUF
        ident = w.tile([128, 128], F32)
        make_identity(nc, ident[:])
        ones1 = w.tile([1, 64], F32)
        nc.vector.memset(ones1[:], 1.0)
        wt = {}
        for name, shape in inp:
            if name in ("efp", "x", "xrows", "am"):
                continue
            wt[name] = w.tile(shape, F32, name=name)
            nc.sync.dma_start(out=wt[name][:], in_=t[name][:])
        idxj = w.tile([128, NG * (TG // 16)], mybir.dt.int16)
        nc.sync.dma_start(out=idxj[:], in_=t["idxj"][:])
        idxe = w.tile([128, NG * (TG // 16)], mybir.dt.int16)
        nc.sync.dma_start(out=idxe[:], in_=t["idxe"][:])

        # ---- stage 0: per-node precomputes for this core's rows
        xr = w.tile([128, 2, C], F32)   # xrows row-major, chunks of 128/32
        nc.sync.dma_start(out=xr[:, 0, :], in_=t["xrows"][0:128, :])
        nc.sync.dma_start(out=xr[:32, 1, :], in_=t["xrows"][128:160, :])
        xrf = w.tile([C, RPC], F32)     # xrows feature-major
        p0 = pst.tile([C, CHUNK], F32, name="ptr")
        nc.tensor.transpose(p0[:, :128], xr[:, 0, :], ident[:])
        nc.tensor.transpose(p0[:, 128:160], xr[:32, 1, :], ident[:32, :32])
        nc.vector.tensor_copy(out=xrf[:], in_=p0[:, :RPC])
        axi = w.tile([C, RPC], F32)
        pa = pst.tile([C, CHUNK], F32, name="ptr")
        nc.tensor.matmul(pa[:, :RPC], wt["Wxi"][:], xrf[:], start=True, stop=True)
        nc.vector.tensor_copy(out=axi[:], in_=pa[:, :RPC])
        selff = w.tile([C, RPC], F32)
        pb = pst.tile([C, CHUNK], F32, name="ptr")
        nc.tensor.matmul(pb[:, :RPC], wt["Ws"][:], xrf[:], start=True, stop=True)
        nc.scalar.activation(selff[:], pb[:, :RPC], AF.Identity, bias=wt["bs"][:])

        IC = TG // 16   # idx columns per group
        for g in range(NG):
            # ---- gathers (token-major: token t = c*128+p -> [p, c, :])
            xj_tm = big.tile([128, NCOL, C], F32, name="xj_tm")
            nc.gpsimd.dma_gather(
                out_ap=xj_tm[:], in_ap=t["x"][:],
                idxs_ap=idxj[:, g * IC:(g + 1) * IC],
                num_idxs=TG, num_idxs_reg=TG, elem_size=C,
                single_packet=False)
            ef_tm = big.tile([128, NCOL, 64], F32, name="ef_tm")
            nc.gpsimd.dma_gather(
                out_ap=ef_tm[:], in_ap=t["efp"][g * RG * N:(g + 1) * RG * N, :],
                idxs_ap=idxe[:, g * IC:(g + 1) * IC],
                num_idxs=TG, num_idxs_reg=TG, elem_size=64,
                single_packet=False)

            # ---- transpose to feature-major
            xj_fm = big.tile([C, TG], F32, name="xj_fm")
            ef_fm = big.tile([E, TG], F32, name="ef_fm")
            for cb in range(NCH):
                ptx = pst.tile([C, CHUNK], F32, name="ptr")
                pte = pst.tile([64, CHUNK], F32, name="pte")
                for k in range(4):
                    c = cb * 4 + k
                    nc.tensor.transpose(ptx[:, k * 128:(k + 1) * 128],
                                        xj_tm[:, c, :], ident[:])
                    nc.tensor.transpose(pte[:, k * 128:(k + 1) * 128],
                                        ef_tm[:, c, :], ident[:])
                cols = slice(cb * CHUNK, (cb + 1) * CHUNK)
                nc.vector.tensor_copy(out=xj_fm[:, cols], in_=ptx[:])
                nc.scalar.activation(ef_fm[:, cols], pte[:E, :], AF.Copy)

            # ---- per-edge MLPs (feature-major, chunks of 512 tokens)
            pe1 = big.tile([64, TG], F32, name="pe1")
            pe2 = big.tile([64, TG], F32, name="pe2")
            pe3 = big.tile([32, TG], F32, name="pe3")
            hg = big.tile([128, TG], F32, name="hg")
            tn = big.tile([64, TG], F32, name="tn")
            h2 = big.tile([32, TG], F32, name="h2")
            gates = big.tile([64, TG], F32, name="gates")
            for q in range(NCH):
                cols = slice(q * CHUNK, (q + 1) * CHUNK)
                ps1 = ps.tile([128, CHUNK], F32, name="mlp")
                nc.tensor.matmul(ps1[:64, :], wt["We1"][:], ef_fm[:, cols],
                                 start=True, stop=True)
                nc.vector.tensor_scalar(out=pe1[:, cols], in0=ps1[:64, :],
                                        scalar1=wt["be1"][:], scalar2=0.0,
                                        op0=OP.add, op1=OP.max)
                ps2 = ps.tile([128, CHUNK], F32, name="mlp")
                nc.tensor.matmul(ps2[:64, :], wt["We2"][:], pe1[:, cols],
                                 start=True, stop=True)
                nc.scalar.activation(pe2[:, cols], ps2[:64, :], AF.Relu,
                                     bias=wt["be2"][:])
                ps3 = ps.tile([128, CHUNK], F32, name="mlp")
                nc.tensor.matmul(ps3[:32, :], wt["We3"][:], pe2[:, cols],
                                 start=True, stop=True)
                nc.vector.tensor_scalar(out=pe3[:, cols], in0=ps3[:32, :],
                                        scalar1=wt["be3"][:], scalar2=0.0,
                                        op0=OP.add, op1=OP.max)
                # h/g joint first layer: Wpe.T@pe3 + Wjj.T@xj (+ axi on rows 0:64)
                ps4 = ps.tile([128, CHUNK], F32, name="mlp")
                nc.tensor.matmul(ps4[:], wt["Wpe"][:], pe3[:, cols],
                                 start=True, stop=False)
                nc.tensor.matmul(ps4[:], wt["Wjj"][:], xj_fm[:, cols],
                                 start=False, stop=True)
                axi_b = bass.AP(
                    tensor=axi.tensor,
                    offset=axi[:, g * RG:(g + 1) * RG].offset,
                    ap=[axi[:].ap[0], [0, CHUNK // RG], [1, RG]])
                nc.vector.scalar_tensor_tensor(
                    out=ps4[:64, :], in0=ps4[:64, :], scalar=0.0, in1=axi_b,
                    op0=OP.add, op1=OP.add)
                nc.scalar.activation(hg[:, cols], ps4[:], AF.Relu,
                                     bias=wt["bhg"][:])
                # tn = Wn.T@xj + bn
                ps5 = ps.tile([128, CHUNK], F32, name="mlp")
                nc.tensor.matmul(ps5[:64, :], wt["Wn"][:], xj_fm[:, cols],
                                 start=True, stop=True)
                nc.scalar.activation(tn[:, cols], ps5[:64, :], AF.Identity,
                                     bias=wt["bn"][:])
                # second layer of attention (h2) and gates (g2), block-diag
                ps6 = ps.tile([128, CHUNK], F32, name="mlp")
                nc.tensor.matmul(ps6[:], wt["W22"][:], hg[:, cols],
                                 start=True, stop=True)
                nc.vector.tensor_scalar(out=h2[:, cols], in0=ps6[:32, :],
                                        scalar1=wt["ba2"][:], scalar2=0.0,
                                        op0=OP.add, op1=OP.max)
                nc.scalar.activation(gates[:, cols], ps6[64:128, :], AF.Sigmoid,
                                     bias=wt["bg2"][:])

            # ---- attention scores, row-major [RG, D]
            psc = pss.tile([RG, D], F32, name="sp")
            for d in range(D):
                nc.tensor.matmul(psc[:, d:d + 1],
                                 h2[:, d * RG:(d + 1) * RG],
                                 wt["Wa3"][:], start=True, stop=True)
            amg = sm.tile([RG, D], F32, name="amg")
            nc.sync.dma_start(out=amg[:], in_=t["am"][g * RG:(g + 1) * RG, :])
            smg = sm.tile([RG, D], F32, name="smg")
            nc.vector.tensor_tensor(out=smg[:], in0=psc[:], in1=amg[:], op=OP.add)
            negmax = sm.tile([RG, 1], F32, name="negmax")
            nc.vector.tensor_reduce(out=negmax[:], in_=smg[:],
                                    axis=mybir.AxisListType.X, op=OP.max,
                                    negate=True)
            pexp = sm.tile([RG, D + 1], F32, name="pexp")
            nc.scalar.activation(pexp[:, :D], smg[:], AF.Exp, bias=negmax[:],
                                 accum_out=pexp[:, D:D + 1])
            invz = sm.tile([RG, 1], F32, name="invz")
            nc.vector.tensor_scalar_add(out=invz[:], in0=pexp[:, D:D + 1],
                                        scalar1=1e-30)
            nc.vector.reciprocal(out=invz[:], in_=invz[:])
            wrow = sm.tile([RG, D], F32, name="wrow")
            nc.vector.tensor_scalar_mul(out=wrow[:], in0=pexp[:, :D],
                                        scalar1=invz[:])
            pwt = pss.tile([D, RG], F32, name="sp")
            nc.tensor.transpose(pwt[:], wrow[:], ident[:RG, :RG])
            wT = sm.tile([D, RG], F32, name="wT")
            nc.vector.tensor_copy(out=wT[:], in_=pwt[:])
            # flatten [D, RG] across partitions into one row [1, TG] (d-major)
            wflat = sm.tile([1, TG], F32, name="wflat")
            nc.sync.dma_start(out=wflat[:], in_=wT[:])

            # ---- weighted messages
            mdw = big.tile([64, TG], F32, name="mdw")
            for u in range(NCH):
                cols = slice(u * CHUNK, (u + 1) * CHUNK)
                pwb = pst.tile([64, CHUNK], F32, name="pwb")
                nc.tensor.matmul(pwb[:], ones1[:], wflat[:, cols],
                                 start=True, stop=True)
                nc.vector.tensor_tensor(out=mdw[:, cols], in0=gates[:, cols],
                                        in1=tn[:, cols], op=OP.mult)
                nc.vector.tensor_tensor(out=mdw[:, cols], in0=mdw[:, cols],
                                        in1=pwb[:], op=OP.mult)
            if debug and g == 0:
                nc.sync.dma_start(out=t["dbg_xj"][:], in_=xj_tm[:])
                nc.sync.dma_start(out=t["dbg_ef"][:], in_=ef_tm[:])
                nc.sync.dma_start(out=t["dbg_wflat"][:], in_=wflat[:])
                nc.sync.dma_start(out=t["dbg_mdw"][:], in_=mdw[:])
                nc.sync.dma_start(out=t["dbg_hg"][:], in_=hg[:])
                nc.sync.dma_start(out=t["dbg_wrow"][:], in_=wrow[:])
            comb = sm.tile([128, RG], F32, name="comb")
            nc.scalar.activation(comb[:64, :], selff[:, g * RG:(g + 1) * RG],
                                 AF.Copy)
            mdw_v = mdw[:].rearrange("p (d r) -> p r d", d=D)
            nc.vector.tensor_reduce(out=comb[64:128, :], in_=mdw_v,
                                    axis=mybir.AxisListType.X, op=OP.add)

            # ---- output MLP + transpose back to row-major
            pc1 = pss.tile([64, RG], F32, name="sp")
            nc.tensor.matmul(pc1[:], wt["Wc1"][:], comb[:], start=True, stop=True)
            c1 = sm.tile([64, RG], F32, name="c1")
            nc.scalar.activation(c1[:], pc1[:], AF.Relu, bias=wt["bc1"][:])
            pc2 = pss.tile([64, RG], F32, name="sp")
            nc.tensor.matmul(pc2[:], wt["Wc2"][:], c1[:], start=True, stop=True)
            ofm = sm.tile([64, RG], F32, name="ofm")
            nc.scalar.activation(ofm[:], pc2[:], AF.Identity, bias=wt["bc2"][:])
            por = pss.tile([RG, 64], F32, name="sp")
            nc.tensor.transpose(por[:], ofm[:], ident[:64, :64])
            orow = sm.tile([RG, 64], F32, name="orow")
            nc.vector.tensor_copy(out=orow[:], in_=por[:])
            nc.sync.dma_start(out=t["out"][g * RG:(g + 1) * RG, :], in_=orow[:])
    nc.compile()
    return nc


_NC = None


def _host_prep(x, adjacency, edge_features, weights):
    """Build per-core input maps."""
    adj = adjacency > 0
    Bn, Nn = adj.shape[0], adj.shape[1]
    # neighbor lists: stable argsort of ~mask puts nonzero-j first, in order
    order = np.argsort(~adj, axis=-1, kind="stable")   # [B, N, N]
    deg = adj.sum(-1)                                  # [B, N]
    assert deg.max() <= D, f"degree {deg.max()} exceeds {D} slots"
    jidx = order[:, :, :D].astype(np.int32)            # [B, N, D]
    slot = np.arange(D)[None, None, :]
    valid = slot < deg[:, :, None]
    jidx = np.where(valid, jidx, 0)
    am = np.where(valid, 0.0, -1e30).astype(np.float32)  # [B, N, D]

    Wa1, Wg1 = weights["Wa1"], weights["Wg1"]
    W22 = np.zeros((128, 128), np.float32)
    W22[:64, :32] = weights["Wa2"]
    W22[64:, 64:] = weights["Wg2"]
    wts = {
        "We1": weights["We1"], "We2": weights["We2"], "We3": weights["We3"],
        "Wpe": np.concatenate([Wa1[2 * C:], Wg1[C:]], 1),
        "Wjj": np.concatenate([Wa1[C:2 * C], Wg1[:C]], 1),
        "Wxi": Wa1[:C], "Wn": weights["Wn"], "W22": W22,
        "Wa3": weights["Wa3"], "Ws": weights["Ws"],
        "Wc1": weights["Wc1"], "Wc2": weights["Wc2"],
        "be1": weights["be1"][:, None], "be2": weights["be2"][:, None],
        "be3": weights["be3"][:, None],
        "bhg": np.concatenate([weights["ba1"], weights["bg1"]])[:, None],
        "bn": weights["bn"][:, None], "ba2": weights["ba2"][:, None],
        "bg2": weights["bg2"][:, None], "bs": weights["bs"][:, None],
        "bc1": weights["bc1"][:, None], "bc2": weights["bc2"][:, None],
    }
    wts = {k: np.ascontiguousarray(v, np.float32) for k, v in wts.items()}

    in_maps = []
    for core in range(NCORES):
        b = core // 4
        i0 = (core % 4) * RPC
        m = dict(wts)
        m["x"] = np.ascontiguousarray(x[b], np.float32)
        m["xrows"] = np.ascontiguousarray(x[b, i0:i0 + RPC], np.float32)
        efp = np.zeros((RPC * N, 64), np.float32)
        efp[:, :E] = edge_features[b, i0:i0 + RPC].reshape(-1, E)
        m["efp"] = efp
        m["am"] = np.zeros((RPC, D), np.float32)
        IC = TG // 16
        ij = np.zeros((128, NG * IC), np.int16)
        ie = np.zeros((128, NG * IC), np.int16)
        for g in range(NG):
            lr = np.arange(g * RG, (g + 1) * RG)
            m["am"][lr] = am[b, i0 + lr]
            jv = jidx[b, i0 + lr]          # [RG, D]
            # token t = d*RG + rr (gather writes token t to [t%128, t//128])
            jvec = np.zeros(TG, np.int64)
            evec = np.zeros(TG, np.int64)
            for d in range(D):
                tt = d * RG + np.arange(RG)
                jvec[tt] = jv[:, d]
                evec[tt] = (lr - g * RG) * N + jv[:, d]   # group-local row
            # wrapped int16 layout: idx[i%16, i//16], replicated over 8 blocks
            assert evec.max() < 32768
            wj = jvec.reshape(IC, 16).T.astype(np.int16)
            we = evec.reshape(IC, 16).T.astype(np.int16)
            ij[:, g * IC:(g + 1) * IC] = np.tile(wj, (8, 1))
            ie[:, g * IC:(g + 1) * IC] = np.tile(we, (8, 1))
        m["idxj"] = ij
        m["idxe"] = ie
        in_maps.append(m)
    return in_maps


def kernel(**inputs):
    global _NC
    x = np.asarray(inputs["x"], np.float32)
    adjacency = np.asarray(inputs["adjacency"], np.float32)
    edge_features = np.asarray(inputs["edge_features"], np.float32)
    weights = {k: np.asarray(v, np.float32) for k, v in inputs.items()
               if k not in ("x", "adjacency", "edge_features")}
    in_maps = _host_prep(x, adjacency, edge_features, weights)
    if _NC is None:
        _NC = _build_nc()
    res = run_bass_kernel_spmd(_NC, in_maps, list(range(NCORES)))
    out = np.zeros((B, N, O), np.float32)
    for core in range(NCORES):
        b = core // 4
        i0 = (core % 4) * RPC
        out[b, i0:i0 + RPC] = res.results[core]["out"]
    return out


# revision 11
# speedup vs baseline: 13.4540x; 13.4540x over previous
"""Trainium2 Bass kernel for nn_EnhancedGraphConv (gnn_message_passing).

Strategy (8 cores): shard the B*N=1280 graph rows (b,i) as 160 rows/core
(cores 0-3 -> batch 0, 4-7 -> batch 1).  Host converts the dense adjacency
into padded neighbor lists (max degree 51 <= 64 slots/row), and the device
gathers only the ~5% of edge_features / x rows that are actually referenced,
via indirect DMA from HBM.  All per-edge MLPs run on compacted tokens in
feature-major layout; attention softmax runs row-major (rows on partitions,
64 neighbor slots on the free dim); messages are reduced with a PE ones-
broadcast + strided DVE reduction.
"""
import numpy as np
from contextlib import ExitStack

import concourse.bass as bass
import concourse.bacc as bacc
import concourse.tile as tile
from concourse import mybir
from concourse.bass_utils import run_bass_kernel_spmd
from concourse.masks import make_identity

F32 = mybir.dt.float32
I32 = mybir.dt.int32
AF = mybir.ActivationFunctionType
OP = mybir.AluOpType

B, N, C, O, E = 2, 640, 64, 64, 18
D = 64            # neighbor slots per row
RG = 32           # rows per group
NCORES = 8
RPC = (B * N) // NCORES   # 160 rows per core
NG = RPC // RG            # 5 groups
TG = D * RG               # 2048 tokens per group
CHUNK = 512               # matmul moving chunk
NCH = TG // CHUNK         # 4 chunks per group
NCOL = TG // 128          # 16 gather columns per group


def _build_nc(debug=False, stage=6):
    nc = bacc.Bacc("TRN2", target_bir_lowering=False)
    t = {}
    inp = [
        ("efp", [RPC * N, 64]), ("x", [N, C]), ("xrows", [RPC, C]),
        ("am", [RPC, D]),
        ("We1", [E, 64]), ("We2", [64, 64]), ("We3", [64, 32]),
        ("Wpe", [32, 128]), ("Wjj", [64, 128]), ("Wxi", [64, 64]),
        ("Wn", [64, 64]), ("W22", [128, 128]), ("Wa3", [32, 1]),
        ("Ws", [64, 64]), ("Wc1", [128, 64]), ("Wc2", [64, 64]),
        ("be1", [64, 1]), ("be2", [64, 1]), ("be3", [32, 1]),
        ("bhg", [128, 1]), ("bn", [64, 1]), ("ba2", [32, 1]),
        ("bg2", [64, 1]), ("bs", [64, 1]), ("bc1", [64, 1]), ("bc2", [64, 1]),
    ]
    for name, shape in inp:
        t[name] = nc.dram_tensor(name, shape, F32, kind="ExternalInput")
    I16 = mybir.dt.int16
    t["idxj"] = nc.dram_tensor("idxj", [128, NG * (TG // 16)], I16, kind="ExternalInput")
    t["idxe"] = nc.dram_tensor("idxe", [128, NG * (TG // 16)], I16, kind="ExternalInput")
    t["out"] = nc.dram_tensor("out", [RPC, O], F32, kind="ExternalOutput")
    if debug:
        t["dbg_xj"] = nc.dram_tensor("dbg_xj", [128, NCOL, C], F32, kind="ExternalOutput")
        t["dbg_ef"] = nc.dram_tensor("dbg_ef", [128, NCOL, 64], F32, kind="ExternalOutput")
        t["dbg_wflat"] = nc.dram_tensor("dbg_wflat", [1, TG], F32, kind="ExternalOutput")
        t["dbg_mdw"] = nc.dram_tensor("dbg_mdw", [64, TG], F32, kind="ExternalOutput")
        t["dbg_hg"] = nc.dram_tensor("dbg_hg", [128, TG], F32, kind="ExternalOutput")
        t["dbg_wrow"] = nc.dram_tensor("dbg_wrow", [RG, D], F32, kind="ExternalOutput")

    with tile.TileContext(nc) as tc, ExitStack() as ctx:
        w = ctx.enter_context(tc.tile_pool(name="w", bufs=1))
        big = ctx.enter_context(tc.tile_pool(name="big", bufs=1))
        sm = ctx.enter_context(tc.tile_pool(name="sm", bufs=2))
        ps = ctx.enter_context(tc.tile_pool(name="ps", bufs=3, space="PSUM"))
        pst = ctx.enter_context(tc.tile_pool(name="pst", bufs=1, space="PSUM"))
        pss = ctx.enter_context(tc.tile_pool(name="pss", bufs=2, space="PSUM"))

        # ---- constants / weights in SBUF
        ident = w.tile([128, 128], F32)
        make_identity(nc, ident[:])
        ones1 = w.tile([1, 64], F32)
        nc.vector.memset(ones1[:], 1.0)
        wt = {}
        for name, shape in inp:
            if name in ("efp", "x", "xrows", "am"):
                continue
            wt[name] = w.tile(shape, F32, name=name)
            nc.sync.dma_start(out=wt[name][:], in_=t[name][:])
        idxj = w.tile([128, NG * (TG // 16)], mybir.dt.int16)
        nc.sync.dma_start(out=idxj[:], in_=t["idxj"][:])
        idxe = w.tile([128, NG * (TG // 16)], mybir.dt.int16)
        nc.sync.dma_start(out=idxe[:], in_=t["idxe"][:])

        # ---- stage 0: per-node precomputes for this core's rows
        xr = w.tile([128, 2, C], F32)   # xrows row-major, chunks of 128/32
        nc.sync.dma_start(out=xr[:, 0, :], in_=t["xrows"][0:128, :])
        nc.sync.dma_start(out=xr[:32, 1, :], in_=t["xrows"][128:160, :])
        xrf = w.tile([C, RPC], F32)     # xrows feature-major
        p0 = pst.tile([C, CHUNK], F32, name="ptr")
        nc.tensor.transpose(p0[:, :128], xr[:, 0, :], ident[:])
        nc.tensor.transpose(p0[:, 128:160], xr[:32, 1, :], ident[:32, :32])
        nc.vector.tensor_copy(out=xrf[:], in_=p0[:, :RPC])
        axi = w.tile([C, RPC], F32)
        pa = pst.tile([C, CHUNK], F32, name="ptr")
        nc.tensor.matmul(pa[:, :RPC], wt["Wxi"][:], xrf[:], start=True, stop=True)
        nc.vector.tensor_copy(out=axi[:], in_=pa[:, :RPC])
        selff = w.tile([C, RPC], F32)
        pb = pst.tile([C, CHUNK], F32, name="ptr")
        nc.tensor.matmul(pb[:, :RPC], wt["Ws"][:], xrf[:], start=True, stop=True)
        nc.scalar.activation(selff[:], pb[:, :RPC], AF.Identity, bias=wt["bs"][:])

        IC = TG // 16   # idx columns per group
        if stage < 6:
            dummy = sm.tile([RG, 64], F32, name="dummy")
            nc.vector.memset(dummy[:], 0.0)
            for g in range(NG):
                nc.sync.dma_start(out=t["out"][g * RG:(g + 1) * RG, :], in_=dummy[:])
        for g in range(NG):
            # ---- gathers (token-major: token t = c*128+p -> [p, c, :])
            xj_tm = big.tile([128, NCOL, C], F32, name="xj_tm")
            nc.gpsimd.dma_gather(
                out_ap=xj_tm[:], in_ap=t["x"][:],
                idxs_ap=idxj[:, g * IC:(g + 1) * IC],
                num_idxs=TG, num_idxs_reg=TG, elem_size=C,
                single_packet=False)
            ef_tm = big.tile([128, NCOL, 64], F32, name="ef_tm")
            nc.gpsimd.dma_gather(
                out_ap=ef_tm[:], in_ap=t["efp"][g * RG * N:(g + 1) * RG * N, :],
                idxs_ap=idxe[:, g * IC:(g + 1) * IC],
                num_idxs=TG, num_idxs_reg=TG, elem_size=64,
                single_packet=False)

            if stage < 2:
                continue
            # ---- transpose to feature-major
            xj_fm = big.tile([C, TG], F32, name="xj_fm")
            ef_fm = big.tile([E, TG], F32, name="ef_fm")
            for cb in range(NCH):
                ptx = pst.tile([C, CHUNK], F32, name="ptr")
                pte = pst.tile([64, CHUNK], F32, name="pte")
                for k in range(4):
                    c = cb * 4 + k
                    nc.tensor.transpose(ptx[:, k * 128:(k + 1) * 128],
                                        xj_tm[:, c, :], ident[:])
                    nc.tensor.transpose(pte[:, k * 128:(k + 1) * 128],
                                        ef_tm[:, c, :], ident[:])
                cols = slice(cb * CHUNK, (cb + 1) * CHUNK)
                nc.vector.tensor_copy(out=xj_fm[:, cols], in_=ptx[:])
                nc.scalar.activation(ef_fm[:, cols], pte[:E, :], AF.Copy)

            if stage < 3:
                continue
            # ---- per-edge MLPs (feature-major, chunks of 512 tokens)
            pe1 = big.tile([64, TG], F32, name="pe1")
            pe2 = big.tile([64, TG], F32, name="pe2")
            pe3 = big.tile([32, TG], F32, name="pe3")
            hg = big.tile([128, TG], F32, name="hg")
            tn = big.tile([64, TG], F32, name="tn")
            h2 = big.tile([32, TG], F32, name="h2")
            gates = big.tile([64, TG], F32, name="gates")
            for q in range(NCH):
                cols = slice(q * CHUNK, (q + 1) * CHUNK)
                ps1 = ps.tile([128, CHUNK], F32, name="mlp")
                nc.tensor.matmul(ps1[:64, :], wt["We1"][:], ef_fm[:, cols],
                                 start=True, stop=True)
                nc.vector.tensor_scalar(out=pe1[:, cols], in0=ps1[:64, :],
                                        scalar1=wt["be1"][:], scalar2=0.0,
                                        op0=OP.add, op1=OP.max)
                ps2 = ps.tile([128, CHUNK], F32, name="mlp")
                nc.tensor.matmul(ps2[:64, :], wt["We2"][:], pe1[:, cols],
                                 start=True, stop=True)
                nc.scalar.activation(pe2[:, cols], ps2[:64, :], AF.Relu,
                                     bias=wt["be2"][:])
                ps3 = ps.tile([128, CHUNK], F32, name="mlp")
                nc.tensor.matmul(ps3[:32, :], wt["We3"][:], pe2[:, cols],
                                 start=True, stop=True)
                nc.vector.tensor_scalar(out=pe3[:, cols], in0=ps3[:32, :],
                                        scalar1=wt["be3"][:], scalar2=0.0,
                                        op0=OP.add, op1=OP.max)
                if stage < 4:
                    continue
                # h/g joint first layer: Wpe.T@pe3 + Wjj.T@xj (+ axi on rows 0:64)
                ps4 = ps.tile([128, CHUNK], F32, name="mlp")
                nc.tensor.matmul(ps4[:], wt["Wpe"][:], pe3[:, cols],
                                 start=True, stop=False)
                nc.tensor.matmul(ps4[:], wt["Wjj"][:], xj_fm[:, cols],
                                 start=False, stop=True)
                axi_b = bass.AP(
                    tensor=axi.tensor,
                    offset=axi[:, g * RG:(g + 1) * RG].offset,
                    ap=[axi[:].ap[0], [0, CHUNK // RG], [1, RG]])
                nc.vector.scalar_tensor_tensor(
                    out=ps4[:64, :], in0=ps4[:64, :], scalar=0.0, in1=axi_b,
                    op0=OP.add, op1=OP.add)
                nc.scalar.activation(hg[:, cols], ps4[:], AF.Relu,
                                     bias=wt["bhg"][:])
                # tn = Wn.T@xj + bn
                ps5 = ps.tile([128, CHUNK], F32, name="mlp")
                nc.tensor.matmul(ps5[:64, :], wt["Wn"][:], xj_fm[:, cols],
                                 start=True, stop=True)
                nc.scalar.activation(tn[:, cols], ps5[:64, :], AF.Identity,
                                     bias=wt["bn"][:])
                # second layer of attention (h2) and gates (g2), block-diag
                ps6 = ps.tile([128, CHUNK], F32, name="mlp")
                nc.tensor.matmul(ps6[:], wt["W22"][:], hg[:, cols],
                                 start=True, stop=True)
                nc.vector.tensor_scalar(out=h2[:, cols], in0=ps6[:32, :],
                                        scalar1=wt["ba2"][:], scalar2=0.0,
                                        op0=OP.add, op1=OP.max)
                nc.scalar.activation(gates[:, cols], ps6[64:128, :], AF.Sigmoid,
                                     bias=wt["bg2"][:])

            if stage < 5:
                continue
            # ---- attention scores, row-major [RG, D]
            psc = pss.tile([RG, D], F32, name="sp")
            for d in range(D):
                nc.tensor.matmul(psc[:, d:d + 1],
                                 h2[:, d * RG:(d + 1) * RG],
                                 wt["Wa3"][:], start=True, stop=True)
            amg = sm.tile([RG, D], F32, name="amg")
            nc.sync.dma_start(out=amg[:], in_=t["am"][g * RG:(g + 1) * RG, :])
            smg = sm.tile([RG, D], F32, name="smg")
            nc.vector.tensor_tensor(out=smg[:], in0=psc[:], in1=amg[:], op=OP.add)
            negmax = sm.tile([RG, 1], F32, name="negmax")
            nc.vector.tensor_reduce(out=negmax[:], in_=smg[:],
                                    axis=mybir.AxisListType.X, op=OP.max,
                                    negate=True)
            pexp = sm.tile([RG, D + 1], F32, name="pexp")
            nc.scalar.activation(pexp[:, :D], smg[:], AF.Exp, bias=negmax[:],
                                 accum_out=pexp[:, D:D + 1])
            invz = sm.tile([RG, 1], F32, name="invz")
            nc.vector.tensor_scalar_add(out=invz[:], in0=pexp[:, D:D + 1],
                                        scalar1=1e-30)
            nc.vector.reciprocal(out=invz[:], in_=invz[:])
            wrow = sm.tile([RG, D], F32, name="wrow")
            nc.vector.tensor_scalar_mul(out=wrow[:], in0=pexp[:, :D],
                                        scalar1=invz[:])
            pwt = pss.tile([D, RG], F32, name="sp")
            nc.tensor.transpose(pwt[:], wrow[:], ident[:RG, :RG])
            wT = sm.tile([D, RG], F32, name="wT")
            nc.vector.tensor_copy(out=wT[:], in_=pwt[:])
            # flatten [D, RG] across partitions into one row [1, TG] (d-major)
            wflat = sm.tile([1, TG], F32, name="wflat")
            nc.sync.dma_start(out=wflat[:], in_=wT[:])

            if stage < 6:
                continue
            # ---- weighted messages
            mdw = big.tile([64, TG], F32, name="mdw")
            for u in range(NCH):
                cols = slice(u * CHUNK, (u + 1) * CHUNK)
                pwb = pst.tile([64, CHUNK], F32, name="pwb")
                nc.tensor.matmul(pwb[:], ones1[:], wflat[:, cols],
                                 start=True, stop=True)
                nc.vector.tensor_tensor(out=mdw[:, cols], in0=gates[:, cols],
                                        in1=tn[:, cols], op=OP.mult)
                nc.vector.tensor_tensor(out=mdw[:, cols], in0=mdw[:, cols],
                                        in1=pwb[:], op=OP.mult)
            if debug and g == 0:
                nc.sync.dma_start(out=t["dbg_xj"][:], in_=xj_tm[:])
                nc.sync.dma_start(out=t["dbg_ef"][:], in_=ef_tm[:])
                nc.sync.dma_start(out=t["dbg_wflat"][:], in_=wflat[:])
                nc.sync.dma_start(out=t["dbg_mdw"][:], in_=mdw[:])
                nc.sync.dma_start(out=t["dbg_hg"][:], in_=hg[:])
                nc.sync.dma_start(out=t["dbg_wrow"][:], in_=wrow[:])
            comb = sm.tile([128, RG], F32, name="comb")
            nc.scalar.activation(comb[:64, :], selff[:, g * RG:(g + 1) * RG],
                                 AF.Copy)
            mdw_v = mdw[:].rearrange("p (d r) -> p r d", d=D)
            nc.vector.tensor_reduce(out=comb[64:128, :], in_=mdw_v,
                                    axis=mybir.AxisListType.X, op=OP.add)

            # ---- output MLP + transpose back to row-major
            pc1 = pss.tile([64, RG], F32, name="sp")
            nc.tensor.matmul(pc1[:], wt["Wc1"][:], comb[:], start=True, stop=True)
            c1 = sm.tile([64, RG], F32, name="c1")
            nc.scalar.activation(c1[:], pc1[:], AF.Relu, bias=wt["bc1"][:])
            pc2 = pss.tile([64, RG], F32, name="sp")
            nc.tensor.matmul(pc2[:], wt["Wc2"][:], c1[:], start=True, stop=True)
            ofm = sm.tile([64, RG], F32, name="ofm")
            nc.scalar.activation(ofm[:], pc2[:], AF.Identity, bias=wt["bc2"][:])
            por = pss.tile([RG, 64], F32, name="sp")
            nc.tensor.transpose(por[:], ofm[:], ident[:64, :64])
            orow = sm.tile([RG, 64], F32, name="orow")
            nc.vector.tensor_copy(out=orow[:], in_=por[:])
            nc.sync.dma_start(out=t["out"][g * RG:(g + 1) * RG, :], in_=orow[:])
    nc.compile()
    return nc


_NC = None


def _host_prep(x, adjacency, edge_features, weights):
    """Build per-core input maps."""
    adj = adjacency > 0
    Bn, Nn = adj.shape[0], adj.shape[1]
    # neighbor lists: stable argsort of ~mask puts nonzero-j first, in order
    order = np.argsort(~adj, axis=-1, kind="stable")   # [B, N, N]
    deg = adj.sum(-1)                                  # [B, N]
    assert deg.max() <= D, f"degree {deg.max()} exceeds {D} slots"
    jidx = order[:, :, :D].astype(np.int32)            # [B, N, D]
    slot = np.arange(D)[None, None, :]
    valid = slot < deg[:, :, None]
    jidx = np.where(valid, jidx, 0)
    am = np.where(valid, 0.0, -1e30).astype(np.float32)  # [B, N, D]

    Wa1, Wg1 = weights["Wa1"], weights["Wg1"]
    W22 = np.zeros((128, 128), np.float32)
    W22[:64, :32] = weights["Wa2"]
    W22[64:, 64:] = weights["Wg2"]
    wts = {
        "We1": weights["We1"], "We2": weights["We2"], "We3": weights["We3"],
        "Wpe": np.concatenate([Wa1[2 * C:], Wg1[C:]], 1),
        "Wjj": np.concatenate([Wa1[C:2 * C], Wg1[:C]], 1),
        "Wxi": Wa1[:C], "Wn": weights["Wn"], "W22": W22,
        "Wa3": weights["Wa3"], "Ws": weights["Ws"],
        "Wc1": weights["Wc1"], "Wc2": weights["Wc2"],
        "be1": weights["be1"][:, None], "be2": weights["be2"][:, None],
        "be3": weights["be3"][:, None],
        "bhg": np.concatenate([weights["ba1"], weights["bg1"]])[:, None],
        "bn": weights["bn"][:, None], "ba2": weights["ba2"][:, None],
        "bg2": weights["bg2"][:, None], "bs": weights["bs"][:, None],
        "bc1": weights["bc1"][:, None], "bc2": weights["bc2"][:, None],
    }
    wts = {k: np.ascontiguousarray(v, np.float32) for k, v in wts.items()}

    in_maps = []
    for core in range(NCORES):
        b = core // 4
        i0 = (core % 4) * RPC
        m = dict(wts)
        m["x"] = np.ascontiguousarray(x[b], np.float32)
        m["xrows"] = np.ascontiguousarray(x[b, i0:i0 + RPC], np.float32)
        efp = np.zeros((RPC * N, 64), np.float32)
        efp[:, :E] = edge_features[b, i0:i0 + RPC].reshape(-1, E)
        m["efp"] = efp
        m["am"] = np.zeros((RPC, D), np.float32)
        IC = TG // 16
        ij = np.zeros((128, NG * IC), np.int16)
        ie = np.zeros((128, NG * IC), np.int16)
        for g in range(NG):
            lr = np.arange(g * RG, (g + 1) * RG)
            m["am"][lr] = am[b, i0 + lr]
            jv = jidx[b, i0 + lr]          # [RG, D]
            # token t = d*RG + rr (gather writes token t to [t%128, t//128])
            jvec = np.zeros(TG, np.int64)
            evec = np.zeros(TG, np.int64)
            for d in range(D):
                tt = d * RG + np.arange(RG)
                jvec[tt] = jv[:, d]
                evec[tt] = (lr - g * RG) * N + jv[:, d]   # group-local row
            # wrapped int16 layout: idx[i%16, i//16], replicated over 8 blocks
            assert evec.max() < 32768
            wj = jvec.reshape(IC, 16).T.astype(np.int16)
            we = evec.reshape(IC, 16).T.astype(np.int16)
            ij[:, g * IC:(g + 1) * IC] = np.tile(wj, (8, 1))
            ie[:, g * IC:(g + 1) * IC] = np.tile(we, (8, 1))
        m["idxj"] = ij
        m["idxe"] = ie
        in_maps.append(m)
    return in_maps


def kernel(**inputs):
    global _NC
    x = np.asarray(inputs["x"], np.float32)
    adjacency = np.asarray(inputs["adjacency"], np.float32)
    edge_features = np.asarray(inputs["edge_features"], np.float32)
    weights = {k: np.asarray(v, np.float32) for k, v in inputs.items()
               if k not in ("x", "adjacency", "edge_features")}
    in_maps = _host_prep(x, adjacency, edge_features, weights)
    if _NC is None:
        _NC = _build_nc()
    res = run_bass_kernel_spmd(_NC, in_maps, list(range(NCORES)))
    out = np.zeros((B, N, O), np.float32)
    for core in range(NCORES):
        b = core // 4
        i0 = (core % 4) * RPC
        out[b, i0:i0 + RPC] = res.results[core]["out"]
    return out
